# revision 5
# baseline (speedup 1.0000x reference)
"""Trainium2 Bass kernel for the CustomRNN problem.

Reference computation (per time step t over T=1024):
    h = tanh(h @ W2.T + x_t[:, None] @ W1.T + b_h)      # h: [B, H]
    y_t = h @ W3.T                                       # [B, O]

Strategy (data-parallel over batch, 8 cores x 16 rows each):
  * The recurrence runs in TRANSPOSED form on-chip: state is g = h^T with
    layout [H, B_loc] (H=512 -> 4 partition chunks of [128, 16]).  Each
    step the full z^T = W2 @ g + W1 @ x_t^T + b_h lives in ONE PSUM tile
    [128, 4*16] (hidden chunk m in columns m*16..(m+1)*16):
        z[:, m]  = [W1|bh][m].T @ [x_t; 1]            (PE, K=2, start=True)
        z[:, m] += sum_k W2T[k, m].T @ g[k]           (PE, 16 accum matmuls)
        g'       = tanh(z)                            (ONE ACT instruction,
                                                       PSUM -> SBUF hist)
    The single merged activation is the key: ACT instructions carry a
    ~185 ns fixed SBUF-access latency, so 4-per-step (baseline) put
    ~800 ns/step of fixed ACT cost on the serial chain; merged it is
    ~240 ns.  The per-step critical path is then
        ACT busy + ACT ack + sem + 16 PE matmuls + PE drain + sem.
  * g' is written into a double-buffered history buffer hist[p, slot, 64];
    every S=32 steps the y output for a finished window is computed as
        y^T = sum_k W3T[k].T @ hist[:, window, k*16:(k+1)*16]
    as 4 matmuls of [10, 512].  These are issued two-per-step a couple of
    steps *after* the window closes, so they run on the PE inside the
    ~460 ns window where the PE would idle waiting for tanh anyway --
    off the critical chain.  Host code undoes the transpose at the end.
"""

import sys

for _p in ("/opt/trn_rl_repo",):
    if _p not in sys.path:
        sys.path.insert(0, _p)

import numpy as np

import concourse.bacc as bacc
import concourse.bass as bass
import concourse.mybir as mybir
import concourse.tile as tile
from concourse.bass_utils import run_bass_kernel_spmd

# Problem constants (hardcoded per contract).
B, T, H, O = 128, 1024, 512, 10
NCORES = 8
BLOC = B // NCORES        # 16 batch rows per core
P = 128                   # partition dim
KC = H // P               # 4 chunks of the hidden dim
S = 32                    # y-window length (N = S*BLOC = 512 moving cols)
ZW = KC * BLOC            # 64: one step's z^T / g columns

F32 = mybir.dt.float32


def build_nc(t_steps: int = T, mm_dt=mybir.dt.float16, reps: int = 1):
    """Build the single-core Bass program (same program runs SPMD on 8 cores).

    reps > 1 repeats the whole recurrence (identical I/O, multiplied
    compute) — used to measure device execution time differentially,
    cancelling RPC/transfer overhead.
    """
    assert t_steps % S == 0
    nwin = t_steps // S

    nc = bacc.Bacc("TRN2", target_bir_lowering=False)
    # x_aug[0, t*16+b] = x[b, t], x_aug[1, :] = 1.0
    xa_d = nc.dram_tensor("x_aug", [2, t_steps * BLOC], mm_dt, kind="ExternalInput")
    w2t_d = nc.dram_tensor("w2t", [H, H], mm_dt, kind="ExternalInput")
    # waug[0, :] = W1, waug[1, :] = b_h  (both in H-major order)
    wa_d = nc.dram_tensor("waug", [2, H], mm_dt, kind="ExternalInput")
    w3t_d = nc.dram_tensor("w3t", [H, O], mm_dt, kind="ExternalInput")
    yt_d = nc.dram_tensor("yT", [O, t_steps * BLOC], F32, kind="ExternalOutput")

    def slot(t):
        # full-T hist: one slot per step, no reuse -> no WAW dep on the
        # activation, so its single (PE) wait attaches to the instruction
        # and the SEQ pre-decodes it off the critical chain.
        return t

    with tile.TileContext(nc) as tc:
        with (
            tc.tile_pool(name="const", bufs=1) as const,
            tc.tile_pool(name="zpsum", bufs=4, space="PSUM") as zpool,
            tc.tile_pool(name="ypsum", bufs=2, space="PSUM") as ypool,
            tc.tile_pool(name="ysb", bufs=2) as yspool,
        ):
            # --- persistent SBUF tensors -------------------------------
            # W2^T tiles, k-major: chunk (k, m) at columns (k*KC + m)*P
            w2sb = const.tile([P, KC * KC * P], mm_dt, tag="w2sb")
            for k in range(KC):
                nc.sync.dma_start(
                    w2sb[:, k * KC * P : (k + 1) * KC * P],
                    w2t_d[k * P : (k + 1) * P, :],
                )
            w3sb = const.tile([P, KC * O], mm_dt, tag="w3sb")
            for k in range(KC):
                nc.sync.dma_start(
                    w3sb[:, k * O : (k + 1) * O], w3t_d[k * P : (k + 1) * P, :]
                )
            wasb = const.tile([2, H], mm_dt, tag="wasb")
            nc.sync.dma_start(wasb[:], wa_d[:])
            xasb = const.tile([2, t_steps * BLOC], mm_dt, tag="xasb")
            nc.sync.dma_start(xasb[:], xa_d[:])

            # hist[p, t, m*16+b]: one slot per step (full T, no reuse)
            hist = const.tile([P, t_steps, ZW], mm_dt, tag="hist", name="hist")
            zeros = const.tile([P, BLOC], mm_dt, tag="zeros")
            nc.vector.memset(zeros[:], 0.0)

            def emit_y_chunk(w, kk):
                """One k-chunk of window w's y matmul (accumulate in yp)."""
                nc.tensor.matmul(
                    ycur[0][:],
                    w3sb[:, kk * O : (kk + 1) * O],
                    hist[:, w * S : (w + 1) * S, kk * BLOC : (kk + 1) * BLOC],
                    start=(kk == 0),
                    stop=(kk == KC - 1),
                )

            def flush_y(w):
                """Copy finished y window w out of PSUM and DMA to DRAM."""
                ys = yspool.tile([O, S * BLOC], F32, tag="ys")
                nc.vector.tensor_copy(ys[:], ycur[0][:])
                nc.sync.dma_start(
                    yt_d[:, w * S * BLOC : (w + 1) * S * BLOC], ys[:]
                )

            ycur = [None]

            # --- the recurrence ---------------------------------------
            for _rep in range(reps):
              for t in range(t_steps):
                sc = slot(t)
                xa = xasb[:, t * BLOC : (t + 1) * BLOC]
                zp = zpool.tile([P, ZW], F32, tag="zp")
                # start=True pending-zeroes the WHOLE 2KB PSUM bank (zero
                # region = bank), so exactly one start/stop pair brackets all
                # 20 matmuls of the step; first touch of each byte range
                # writes, later touches accumulate.
                # input + bias term first: z[:, m] = [W1|bh][m].T @ [x_t; 1]
                for m in range(KC):
                    nc.tensor.matmul(
                        zp[:, m * BLOC : (m + 1) * BLOC],
                        wasb[:, m * P : (m + 1) * P],
                        xa,
                        start=(m == 0),
                        stop=False,
                    )
                for k in range(KC):
                    if t == 0:
                        rhs = zeros[:]
                    else:
                        pc = slot(t - 1)
                        rhs = hist[:, pc, k * BLOC : (k + 1) * BLOC]
                    for m in range(KC):
                        nc.tensor.matmul(
                            zp[:, m * BLOC : (m + 1) * BLOC],
                            w2sb[:, (k * KC + m) * P : (k * KC + m + 1) * P],
                            rhs,
                            start=False,
                            stop=(k == KC - 1 and m == KC - 1),
                        )

                # y matmuls for the window that closed at t0-1 = S*w+S-1:
                # two per step at t0+2 / t0+3, hidden in the PE's ACT-wait
                # shadow (slots stay valid until window w+2 starts).
                tw = t % S
                if t >= S and tw in (2, 3):
                    w = t // S - 1
                    if tw == 2:
                        ycur[0] = ypool.tile([O, S * BLOC], F32, tag="yp", name="yp")
                        emit_y_chunk(w, 0)
                        emit_y_chunk(w, 1)
                    else:
                        emit_y_chunk(w, 2)
                        emit_y_chunk(w, 3)
                        flush_y(w)

                # g' = tanh(z): ONE merged activation (PSUM -> SBUF hist)
                nc.scalar.activation(
                    hist[:, sc, :],
                    zp[:],
                    mybir.ActivationFunctionType.Tanh,
                )

              # last window of this rep (no later steps to hide behind)
              ycur[0] = ypool.tile([O, S * BLOC], F32, tag="yp", name="yp")
              for kk in range(KC):
                  emit_y_chunk(nwin - 1, kk)
              flush_y(nwin - 1)
    nc.compile()
    return nc


def _np_mm_dtype(mm_dt):
    return {F32: np.float32, mybir.dt.float16: np.float16}[mm_dt]


def make_in_maps(x, W1, W2, W3, b_h, t_steps: int = T, mm_dt=mybir.dt.float16):
    x = np.asarray(x, np.float32)[:, :t_steps]
    W1 = np.asarray(W1, np.float32)
    W2 = np.asarray(W2, np.float32)
    W3 = np.asarray(W3, np.float32)
    b_h = np.asarray(b_h, np.float32)
    mdt = _np_mm_dtype(mm_dt)

    w2t = np.ascontiguousarray(W2.T).astype(mdt)          # [H, H]
    w3t = np.ascontiguousarray(W3.T).astype(mdt)          # [H, O]
    waug = np.stack([W1.reshape(-1), b_h]).astype(mdt)    # [2, H]

    in_maps = []
    for c in range(NCORES):
        xs = x[c * BLOC : (c + 1) * BLOC]                  # [16, t]
        xaug = np.empty((2, t_steps * BLOC), mdt)
        xaug[0] = np.ascontiguousarray(xs.T).reshape(-1)   # (t, b) order
        xaug[1] = 1.0
        in_maps.append({"x_aug": xaug, "w2t": w2t, "w3t": w3t, "waug": waug})
    return in_maps


def gather_output(results, t_steps: int = T):
    out = np.empty((B, t_steps, O), np.float32)
    for c in range(NCORES):
        yt = results[c]["yT"]  # [O, t*16] in (o, t, b) order
        out[c * BLOC : (c + 1) * BLOC] = (
            yt.reshape(O, t_steps, BLOC).transpose(2, 1, 0)
        )
    return out


_NC_CACHE = {}

MM_DT = mybir.dt.float16  # matmul dtype for W2/W3/state (accumulation is f32)


def kernel(x, W1, W2, W3, b_h):
    key = (T, MM_DT)
    if key not in _NC_CACHE:
        _NC_CACHE[key] = build_nc(T, MM_DT)
    nc = _NC_CACHE[key]
    in_maps = make_in_maps(x, W1, W2, W3, b_h, T, MM_DT)
    res = run_bass_kernel_spmd(nc, in_maps, core_ids=list(range(NCORES))).results
    return gather_output(res, T)


# revision 8
# speedup vs baseline: 1.5131x; 1.5131x over previous
"""Trainium2 Bass kernel for the CustomRNN problem.

Reference computation (per time step t over T=1024):
    h = tanh(h @ W2.T + x_t[:, None] @ W1.T + b_h)      # h: [B, H]
    y_t = h @ W3.T                                       # [B, O]

Strategy (data-parallel over batch, 8 cores x 16 rows each):
  * The recurrence runs in TRANSPOSED form on-chip: state is g = h^T with
    layout [H, B_loc] (H=512 -> 4 partition chunks of [128, b]).  The 16
    batch rows per core are split into TWO INDEPENDENT half-chains of 8
    rows that pipeline against each other: while half A waits on its tanh
    (ACT busy + ack + sem ~460 ns), the PE runs half B's matmuls, so the
    per-step critical path of each chain hides the other's.
  * Per half, per step: z^T = W2 @ g + W1 @ x_t^T + b_h accumulates in one
    PSUM tile [128, 4*8] (hidden chunk m at columns m*8..(m+1)*8):
        z[:, m]  = [W1|bh][m].T @ [x_t; 1]      (PE, K=2)
        z[:, m] += sum_k W2T[k, m].T @ g[k]     (PE, 16 accum matmuls, N=8)
        g'       = tanh(z)                      (ONE ACT instr per half,
                                                 PSUM -> SBUF hist)
    One merged ACT per half matters: ACT instructions carry ~185 ns fixed
    SBUF access latency + ~185 ns ack, so fewer/larger ACTs win.  PSUM
    start=True pending-zeroes the whole 2KB bank, so exactly one
    start/stop pair brackets each step's 20 matmuls.
  * hist[p, t, half*32 + m*8 + b] holds ALL T slots (128KB/partition):
    no slot reuse -> no WAW dep on the ACT -> its single PE wait attaches
    to the instruction and the SEQ pre-decodes off the critical chain.
  * Every S=32 steps the y output for a finished window is computed as
        y^T[:, s, half*8+b] = sum_k W3T[k].T @ hist[:, window, ...]
    as 8 matmuls of [10, 256], issued two-per-step a few steps after the
    window closes so they run in the PE's ACT-wait shadow -- off the
    critical chain.  Host code undoes the transpose at the end.
"""

import sys

for _p in ("/opt/trn_rl_repo",):
    if _p not in sys.path:
        sys.path.insert(0, _p)

import numpy as np

import concourse.bacc as bacc
import concourse.bass as bass
import concourse.mybir as mybir
import concourse.tile as tile
from concourse.bass_utils import run_bass_kernel_spmd

# Problem constants (hardcoded per contract).
B, T, H, O = 128, 1024, 512, 10
NCORES = 8
BLOC = B // NCORES        # 16 batch rows per core
HB = BLOC // 2            # 8 rows per half-chain
P = 128                   # partition dim
KC = H // P               # 4 chunks of the hidden dim
S = 32                    # y-window length
ZW = KC * BLOC            # 64 hist columns per step (two halves of 32)
ZH = KC * HB              # 32: one half-step's z^T columns

F32 = mybir.dt.float32


def build_nc(t_steps: int = T, mm_dt=mybir.dt.float16, reps: int = 1):
    """Build the single-core Bass program (same program runs SPMD on 8 cores).

    reps > 1 repeats the whole recurrence (identical I/O, multiplied
    compute) — used to measure device execution time differentially,
    cancelling RPC/transfer overhead.
    """
    assert t_steps % S == 0
    nwin = t_steps // S

    nc = bacc.Bacc("TRN2", target_bir_lowering=False)
    # x_aug[0, t*16+b] = x[b, t], x_aug[1, :] = 1.0
    xa_d = nc.dram_tensor("x_aug", [2, t_steps * BLOC], mm_dt, kind="ExternalInput")
    w2t_d = nc.dram_tensor("w2t", [H, H], mm_dt, kind="ExternalInput")
    # waug[0, :] = W1, waug[1, :] = b_h  (both in H-major order)
    wa_d = nc.dram_tensor("waug", [2, H], mm_dt, kind="ExternalInput")
    w3t_d = nc.dram_tensor("w3t", [H, O], mm_dt, kind="ExternalInput")
    yt_d = nc.dram_tensor("yT", [O, t_steps * BLOC], F32, kind="ExternalOutput")

    with tile.TileContext(nc) as tc:
        with (
            tc.tile_pool(name="const", bufs=1) as const,
            tc.tile_pool(name="zpsum", bufs=2, space="PSUM") as zpool,
            tc.tile_pool(name="ypsum", bufs=1, space="PSUM") as ypool,
            tc.tile_pool(name="ysb", bufs=2) as yspool,
        ):
            # --- persistent SBUF tensors -------------------------------
            # W2^T tiles, k-major: chunk (k, m) at columns (k*KC + m)*P
            w2sb = const.tile([P, KC * KC * P], mm_dt, tag="w2sb")
            for k in range(KC):
                nc.sync.dma_start(
                    w2sb[:, k * KC * P : (k + 1) * KC * P],
                    w2t_d[k * P : (k + 1) * P, :],
                )
            w3sb = const.tile([P, KC * O], mm_dt, tag="w3sb")
            for k in range(KC):
                nc.sync.dma_start(
                    w3sb[:, k * O : (k + 1) * O], w3t_d[k * P : (k + 1) * P, :]
                )
            wasb = const.tile([2, H], mm_dt, tag="wasb")
            nc.sync.dma_start(wasb[:], wa_d[:])
            xasb = const.tile([2, t_steps * BLOC], mm_dt, tag="xasb")
            nc.sync.dma_start(xasb[:], xa_d[:])

            # hist[p, t, half*32 + m*8 + b]: one slot per step, full T
            hist = const.tile([P, t_steps, ZW], mm_dt, tag="hist", name="hist")
            zeros = const.tile([P, HB], mm_dt, tag="zeros")
            nc.vector.memset(zeros[:], 0.0)

            def emit_step_half(t, g):
                """All PE matmuls + the merged tanh for half-chain g, step t."""
                xa = xasb[:, t * BLOC + g * HB : t * BLOC + (g + 1) * HB]
                zp = zpool.tile([P, ZH], F32, tag=f"zp{g}", name=f"zp{g}")
                for m in range(KC):
                    nc.tensor.matmul(
                        zp[:, m * HB : (m + 1) * HB],
                        wasb[:, m * P : (m + 1) * P],
                        xa,
                        start=(m == 0),
                        stop=False,
                    )
                for k in range(KC):
                    if t == 0:
                        rhs = zeros[:]
                    else:
                        rhs = hist[
                            :, t - 1, g * ZH + k * HB : g * ZH + (k + 1) * HB
                        ]
                    for m in range(KC):
                        nc.tensor.matmul(
                            zp[:, m * HB : (m + 1) * HB],
                            w2sb[:, (k * KC + m) * P : (k * KC + m + 1) * P],
                            rhs,
                            start=False,
                            stop=(k == KC - 1 and m == KC - 1),
                        )
                nc.scalar.activation(
                    hist[:, t, g * ZH : (g + 1) * ZH],
                    zp[:],
                    mybir.ActivationFunctionType.Tanh,
                )

            def emit_y_chunk(w, g, kk):
                """One k-chunk of half g of window w's y matmul (into yp)."""
                nc.tensor.matmul(
                    ycur[g][:],
                    w3sb[:, kk * O : (kk + 1) * O],
                    hist[
                        :,
                        w * S : (w + 1) * S,
                        g * ZH + kk * HB : g * ZH + (kk + 1) * HB,
                    ],
                    start=(kk == 0),
                    stop=(kk == KC - 1),
                )

            def flush_y_half(w, g):
                """Copy half g of finished window w out of PSUM, DMA out.
                DRAM column layout per window: w*512 + g*256 + s*8 + b."""
                ys = yspool.tile([O, S * HB], F32, tag=f"ys{g}", name=f"ys{g}")
                nc.vector.tensor_copy(ys[:], ycur[g][:])
                base = w * S * BLOC + g * S * HB
                nc.sync.dma_start(yt_d[:, base : base + S * HB], ys[:])

            ycur = [None, None]

            # --- the recurrence ---------------------------------------
            for _rep in range(reps):
              for t in range(t_steps):
                emit_step_half(t, 0)
                emit_step_half(t, 1)

                # y matmuls for the window that closed at S*w+S-1: two per
                # step at t0+2..t0+5, hidden in the PE's ACT-wait shadow.
                tw = t % S
                if t >= S and tw in (2, 3, 4, 5):
                    w = t // S - 1
                    if tw == 2:
                        ycur[0] = ypool.tile([O, S * HB], F32, tag="yp0", name="yp0")
                        emit_y_chunk(w, 0, 0)
                        emit_y_chunk(w, 0, 1)
                    elif tw == 3:
                        emit_y_chunk(w, 0, 2)
                        emit_y_chunk(w, 0, 3)
                        flush_y_half(w, 0)
                    elif tw == 4:
                        ycur[1] = ypool.tile([O, S * HB], F32, tag="yp1", name="yp1")
                        emit_y_chunk(w, 1, 0)
                        emit_y_chunk(w, 1, 1)
                    else:
                        emit_y_chunk(w, 1, 2)
                        emit_y_chunk(w, 1, 3)
                        flush_y_half(w, 1)

              # last window of this rep (no later steps to hide behind)
              for g in range(2):
                  ycur[g] = ypool.tile(
                      [O, S * HB], F32, tag=f"yp{g}", name=f"yp{g}"
                  )
                  for kk in range(KC):
                      emit_y_chunk(nwin - 1, g, kk)
                  flush_y_half(nwin - 1, g)
    nc.compile()
    return nc


def _np_mm_dtype(mm_dt):
    return {F32: np.float32, mybir.dt.float16: np.float16}[mm_dt]


def make_in_maps(x, W1, W2, W3, b_h, t_steps: int = T, mm_dt=mybir.dt.float16):
    x = np.asarray(x, np.float32)[:, :t_steps]
    W1 = np.asarray(W1, np.float32)
    W2 = np.asarray(W2, np.float32)
    W3 = np.asarray(W3, np.float32)
    b_h = np.asarray(b_h, np.float32)
    mdt = _np_mm_dtype(mm_dt)

    w2t = np.ascontiguousarray(W2.T).astype(mdt)          # [H, H]
    w3t = np.ascontiguousarray(W3.T).astype(mdt)          # [H, O]
    waug = np.stack([W1.reshape(-1), b_h]).astype(mdt)    # [2, H]

    in_maps = []
    for c in range(NCORES):
        xs = x[c * BLOC : (c + 1) * BLOC]                  # [16, t]
        xaug = np.empty((2, t_steps * BLOC), mdt)
        xaug[0] = np.ascontiguousarray(xs.T).reshape(-1)   # (t, b) order
        xaug[1] = 1.0
        in_maps.append({"x_aug": xaug, "w2t": w2t, "w3t": w3t, "waug": waug})
    return in_maps


def gather_output(results, t_steps: int = T):
    nwin = t_steps // S
    out = np.empty((B, t_steps, O), np.float32)
    for c in range(NCORES):
        yt = results[c]["yT"]  # [O, w*512 + g*256 + s*8 + b]
        y = yt.reshape(O, nwin, 2, S, HB).transpose(2, 4, 1, 3, 0)
        out[c * BLOC : (c + 1) * BLOC] = y.reshape(BLOC, t_steps, O)
    return out


_NC_CACHE = {}

MM_DT = mybir.dt.float16  # matmul dtype for W2/W3/state (accumulation is f32)


def kernel(x, W1, W2, W3, b_h):
    key = (T, MM_DT)
    if key not in _NC_CACHE:
        _NC_CACHE[key] = build_nc(T, MM_DT)
    nc = _NC_CACHE[key]
    in_maps = make_in_maps(x, W1, W2, W3, b_h, T, MM_DT)
    res = run_bass_kernel_spmd(nc, in_maps, core_ids=list(range(NCORES))).results
    return gather_output(res, T)
